# revision 25
# baseline (speedup 1.0000x reference)
"""3x3 grayscale dilation (all-ones SE) = 3x3 max-filter, stride 1, zero pad.

Input (8, 3, 1024, 1024) f32 -> same-shape output. 24 images, 3 per core.

fp16 end-to-end (rel tol 2e-2 >> fp16 rounding ~5e-4): halves DMA traffic
(12.6 MB/core vs the ~360 GB/s/core bus) and doubles DVE throughput (2-byte
packed operands hit 2x_1p; measured 0.54-0.56 ns/elem on HW).

Device layout per image: [128 partitions, 8 rows, 2 (even|odd cols), 512];
host pre-packs (cast + column deinterleave + partition-major) so every DMA is
a contiguous >=4 KiB-per-partition descriptor run. Horizontal pair trick is
unit-stride:
    hp   = max(E, O); hmE = max(O[j-1], hp[j]); hmO = max(hp[j], E[j+1])
    (borders max(hp, 0) as small DVE tensor_scalar ops)
hm lands in rows 1..8 of a 10-row tile whose rows 0/9 are partition-halo rows
(zero padding at image edges), then vertical is a pure pair trick:
    vp[q] = max(hmx[2q], hmx[2q+1]);  vm[2s] = max(vp[s], hmx[2s+2]);
    vm[2s+1] = max(hmx[2s+1], vp[s+1])
Halos either via PE shift-matmul -> PSUM -> ACT copy (HALO_DMA=False) or via
an SBUF->SBUF partition-shifted DMA on the otherwise-idle sync ring
(HALO_DMA=True; partition 0/127 halo rows zeroed by gpsimd).

Measured scheduling facts baked in: first DMA bytes land ~8.1 us regardless
of queue (global bring-up); the scalar (ACT) HWDGE ring sustains ~330 GB/s
vs sync's ~190, so all loads ride scalar with image 0 in three row-chunks
(DVE starts ~12 us on the first 512 KiB); emission interleaves image i's
vertical stage after image i+1's horizontal stage so halo latency hides
under DVE work; stores are split (halves, last image quarters) so the final
store tail is ~1.5 us.
"""

import sys

sys.path.insert(0, "/opt/trn_rl_repo")

import numpy as np

N_CORES = 8
IMGS_PER_CORE = 3
H = W = 1024
R = 8  # rows per partition
P = 128
J = 512  # half-row (even/odd) length
FREE = R * 2 * J  # 8192 elems per partition per image

IMG0_CHUNKS = [(0, 2), (2, 4), (4, 6), (6, 8)]
HALO_DMA = "--halo-dma" in sys.argv

_COMPILED_NC = None


def _build_nc(halo_dma=None):
    import concourse.mybir as mybir
    import concourse.tile as tile
    from concourse import bacc

    if halo_dma is None:
        halo_dma = HALO_DMA

    f16 = mybir.dt.float16
    f32 = mybir.dt.float32
    MAX = mybir.AluOpType.max

    nc = bacc.Bacc(None)
    x = nc.declare_dram_parameter("input", [IMGS_PER_CORE, P, FREE], f16, isOutput=False)
    y = nc.declare_dram_parameter("output", [IMGS_PER_CORE, P, FREE], f16, isOutput=True)

    with tile.TileContext(nc) as tc:
        with (
            tc.tile_pool(name="io", bufs=6) as io,
            tc.tile_pool(name="tmp", bufs=6) as tmp,
            tc.tile_pool(name="hmp", bufs=3) as hmp,
            tc.tile_pool(name="shp", bufs=1) as shp,
            tc.tile_pool(name="psum", bufs=2, space="PSUM") as psp,
        ):
            # Shifted identities (fp16) on the otherwise-idle Pool engine.
            sdn = shp.tile([P, P], f16, tag="sdn")
            sup = shp.tile([P, P], f16, tag="sup")
            for t, base in ((sdn, 1), (sup, -1)):
                nc.gpsimd.memset(t[:], 0.0)
                nc.gpsimd.affine_select(
                    out=t[:],
                    in_=t[:],
                    compare_op=mybir.AluOpType.not_equal,
                    fill=1.0,
                    base=base,
                    pattern=[[-1, P]],
                    channel_multiplier=1,
                )

            X = [None] * IMGS_PER_CORE
            hmx = [None] * IMGS_PER_CORE

            # ALL loads on the scalar (fast, ~330 GB/s) ring, in consumption
            # order. A second concurrent queue would starve image 0's small
            # chunks (DMA engines round-robin per DESCRIPTOR, so 16 KiB
            # descriptors on another queue get 4x the bytes of these 4 KiB
            # ones). One queue delivers bytes exactly in program order, and
            # the v-stages between h-stages give the ring time to stay ahead.
            for i in range(IMGS_PER_CORE):
                X[i] = io.tile([P, FREE], f16, tag="io", name=f"X{i}")
                if i == 0:
                    for r0, r1 in IMG0_CHUNKS:
                        c0, c1 = r0 * 2 * J, r1 * 2 * J
                        nc.scalar.dma_start(out=X[i][:, c0:c1], in_=x[i][:, c0:c1])
                else:
                    nc.scalar.dma_start(out=X[i][:], in_=x[i][:])

            def h_stage(i, chunks):
                X3 = X[i][:].rearrange("p (r e j) -> p r e j", e=2, j=J)
                hp = tmp.tile([P, R * J], f16, tag="tmp", name=f"hp{i}")
                hp3 = hp[:].rearrange("p (r j) -> p r j", j=J)
                hmx[i] = hmp.tile([P, 10 * 2 * J], f16, tag="hm", name=f"hmx{i}")
                h3 = hmx[i][:].rearrange("p (r e j) -> p r e j", e=2, j=J)
                hmf = hmx[i][:].rearrange("p (r c) -> p r c", c=2 * J)
                hpf = hp[:].rearrange("p (r c) -> p r c", c=J)
                for r0, r1 in chunks:
                    nc.vector.tensor_tensor(
                        out=hp3[:, r0:r1],
                        in0=X3[:, r0:r1, 0],
                        in1=X3[:, r0:r1, 1],
                        op=MAX,
                    )
                    nc.vector.tensor_tensor(
                        out=h3[:, r0 + 1 : r1 + 1, 0, 1:J],
                        in0=X3[:, r0:r1, 1, 0 : J - 1],
                        in1=hp3[:, r0:r1, 1:J],
                        op=MAX,
                    )
                    nc.vector.tensor_tensor(
                        out=h3[:, r0 + 1 : r1 + 1, 1, 0 : J - 1],
                        in0=hp3[:, r0:r1, 0 : J - 1],
                        in1=X3[:, r0:r1, 0, 1:J],
                        op=MAX,
                    )
                    # both zero-pad border columns (E col 0 at flat 0, O col
                    # J-1 at flat 2J-1) in one strided max-with-0 on the idle
                    # Pool engine (ACT would hoist a ~1.3us activation-table
                    # load ahead of the load DMA issues)
                    nc.gpsimd.tensor_scalar_max(
                        out=hmf[:, r0 + 1 : r1 + 1, 0 : 2 * J : 2 * J - 1],
                        in0=hpf[:, r0:r1, 0 : J : J - 1],
                        scalar1=0.0,
                    )

            def halo_stage(i):
                hm = hmx[i]
                row1 = 1 * 2 * J
                row8 = 8 * 2 * J
                row9 = 9 * 2 * J
                if halo_dma:
                    # hmx row 0 of partition p = hm row 8 of partition p-1;
                    # hmx row 9 of partition p = hm row 1 of partition p+1.
                    # Border partitions get the zero pad from the zero tile.
                    nc.sync.dma_start(out=hm[0:1, 0 : 2 * J], in_=zt[0:1, :])
                    nc.sync.dma_start(
                        out=hm[P - 1 : P, row9 : row9 + 2 * J],
                        in_=zt[P - 1 : P, :],
                    )
                    nc.sync.dma_start(
                        out=hm[1:P, 0 : 2 * J],
                        in_=hm[0 : P - 1, row8 : row8 + 2 * J],
                    )
                    nc.sync.dma_start(
                        out=hm[0 : P - 1, row9 : row9 + 2 * J],
                        in_=hm[1:P, row1 : row1 + 2 * J],
                    )
                else:
                    ps = psp.tile([P, 2 * 2 * J], f32, tag="ps")  # [dh | uh]
                    for c0 in (0, J):
                        nc.tensor.matmul(
                            ps[:, 2 * J + c0 : 2 * J + c0 + J],
                            sup[:],
                            hm[:, row1 + c0 : row1 + c0 + J],
                            start=True,
                            stop=True,
                        )
                    for c0 in (0, J):
                        nc.tensor.matmul(
                            ps[:, c0 : c0 + J],
                            sdn[:],
                            hm[:, row8 + c0 : row8 + c0 + J],
                            start=True,
                            stop=True,
                        )
                    nc.scalar.copy(out=hm[:, 0 : 2 * J], in_=ps[:, 0 : 2 * J])
                    nc.scalar.copy(out=hm[:, row9:], in_=ps[:, 2 * J :])

            def v_stage(i, last):
                # vp anchored on interior pairs (1,2)(3,4)(5,6)(7,8): vp needs
                # no halo rows (shorter cross-engine chain) and the vertical
                # stage totals 12288 elems instead of 13312.
                #   vm[2s]   = max(hmx[2s],   vp[s])
                #   vm[2s+1] = max(vp[s], hmx[2s+3])
                # vm writes de-interleaved row blocks (even rows 0,2,4,6 in
                # the first half of the tile, odd rows 1,3,5,7 in the second;
                # host re-interleaves in _unpack). Fully contiguous outputs
                # dodge the DVE write-path penalty measured on 8 KB-strided
                # out APs at count 4 (~2.4x slower than 2x_1p), letting each
                # image's vm run as one full-width op per parity: 8 fewer
                # DVE instructions than per-(0,4)-group pairs.
                h3 = hmx[i][:].rearrange("p (r e j) -> p r e j", e=2, j=J)
                vp = tmp.tile([P, 4 * 2 * J], f16, tag="tmp", name=f"vp{i}")
                vp3 = vp[:].rearrange("p (q e j) -> p q e j", e=2, j=J)
                vm = io.tile([P, FREE], f16, tag="io")
                # [b (even|odd block), q (4 rows), e, j]
                vm4 = vm[:].rearrange("p (b q e j) -> p b q e j", b=2, e=2, j=J)
                nc.vector.tensor_tensor(
                    out=vp3[:], in0=h3[:, 1:9:2], in1=h3[:, 2:10:2], op=MAX
                )
                # even rows: vm[2q] = max(hmx[2q], vp[q]); odd rows:
                # vm[2q+1] = max(vp[q], hmx[2q+3]). One full-width op per
                # parity for every image (stores read slices, so the last
                # image's half-size store granularity is preserved).
                nc.vector.tensor_tensor(
                    out=vm4[:, 0],
                    in0=h3[:, 0:8:2],
                    in1=vp3[:],
                    op=MAX,
                )
                nc.vector.tensor_tensor(
                    out=vm4[:, 1],
                    in0=vp3[:],
                    in1=h3[:, 3:10:2],
                    op=MAX,
                )
                # stores: one contiguous transfer per parity block (halves
                # for the last image; its final half rides scalar after the
                # last DVE op, the sync one fires earlier)
                if not last:
                    eng = "sync" if i == 0 else "scalar"
                    for b in (0, 1):
                        getattr(nc, eng).dma_start(
                            out=y[i][:, b * 4 * 2 * J : (b + 1) * 4 * 2 * J],
                            in_=vm[:, b * 4 * 2 * J : (b + 1) * 4 * 2 * J],
                        )
                else:
                    for (b, q0, q1), eng in (
                        ((0, 0, 2), "scalar"),
                        ((0, 2, 4), "scalar"),
                        ((1, 0, 2), "sync"),
                        ((1, 2, 4), "scalar"),
                    ):
                        c0 = (b * 4 + q0) * 2 * J
                        c1 = (b * 4 + q1) * 2 * J
                        getattr(nc, eng).dma_start(
                            out=y[i][:, c0:c1], in_=vm[:, c0:c1]
                        )

            h_stage(0, IMG0_CHUNKS)
            halo_stage(0)
            v_stage(0, last=False)
            h_stage(1, [(0, 8)])
            halo_stage(1)
            v_stage(1, last=False)
            h_stage(2, [(0, 8)])
            halo_stage(2)
            v_stage(2, last=True)

    nc.compile()
    return nc


def _get_nc():
    global _COMPILED_NC
    if _COMPILED_NC is None:
        _COMPILED_NC = _build_nc()
    return _COMPILED_NC


def _pack(input):
    # (n, H, W) f32 -> (n, P, FREE) f16, per-partition [row][even|odd]
    x16 = input.reshape(-1, P, R, J, 2).astype(np.float16)
    return np.ascontiguousarray(x16.transpose(0, 1, 2, 4, 3)).reshape(-1, P, FREE)


def _unpack(out):
    # (n, P, FREE) f16 -> (n, H, W) f32.
    # Device layout per partition: [even rows 0,2,4,6 | odd rows 1,3,5,7],
    # each row [E cols | O cols]; re-interleave both here.
    o = out.reshape(-1, P, 2, 4, 2, J).transpose(0, 1, 3, 2, 5, 4)
    return np.ascontiguousarray(o).reshape(-1, H, W).astype(np.float32)


def _reference_fallback(input, se):
    se = np.asarray(se, dtype=np.float32)
    se_h, se_w = se.shape
    pad_h, pad_w = se_h // 2, se_w // 2
    se_m1 = (se - 1.0).reshape(-1)
    padded = np.pad(input, ((0, 0), (0, 0), (pad_h, pad_h), (pad_w, pad_w)))
    out = None
    Hh, Ww = input.shape[2], input.shape[3]
    for i in range(se_h * se_w):
        xs, ys = i // se_h, i % se_h
        mask = np.float32(1.0) if se_m1[i] >= 0 else np.float32(0.0)
        contrib = mask * padded[:, :, xs : xs + Hh, ys : ys + Ww] + se_m1[i]
        out = contrib if out is None else np.maximum(out, contrib)
    return out


def kernel(input, se):
    from concourse.bass_utils import run_bass_kernel_spmd

    input = np.ascontiguousarray(np.asarray(input, dtype=np.float32))
    se_np = np.asarray(se, dtype=np.float32)
    if se_np.shape != (3, 3) or not np.all(se_np == 1.0) or input.shape != (
        8,
        3,
        H,
        W,
    ):
        return _reference_fallback(input, se_np).astype(np.float32)

    nc = _get_nc()
    flat = _pack(input.reshape(N_CORES * IMGS_PER_CORE, H, W))
    in_maps = [
        {"input": flat[k * IMGS_PER_CORE : (k + 1) * IMGS_PER_CORE]}
        for k in range(N_CORES)
    ]
    last_err = None
    for _attempt in range(3):
        try:
            res = run_bass_kernel_spmd(nc, in_maps, list(range(N_CORES)))
            out = np.concatenate(
                [res.results[k]["output"] for k in range(N_CORES)], axis=0
            )
            return _unpack(out).reshape(8, 3, H, W)
        except Exception as e:  # transient NRT_EXEC_UNIT_UNRECOVERABLE etc.
            last_err = e
    raise last_err



# revision 27
# speedup vs baseline: 1.0534x; 1.0534x over previous
"""3x3 grayscale dilation (all-ones SE) = 3x3 max-filter, stride 1, zero pad.

Input (8, 3, 1024, 1024) f32 -> same-shape output. 24 images, 3 per core.

fp16 end-to-end (rel tol 2e-2 >> fp16 rounding ~5e-4): halves DMA traffic
(12.6 MB/core vs the ~360 GB/s/core bus) and doubles DVE throughput (2-byte
packed operands hit 2x_1p; measured 0.54-0.56 ns/elem on HW).

Device layout per image: [128 partitions, 8 rows, 2 (even|odd cols), 512];
host pre-packs (cast + column deinterleave + partition-major) so every DMA is
a contiguous >=4 KiB-per-partition descriptor run. Horizontal pair trick is
unit-stride:
    hp   = max(E, O); hmE = max(O[j-1], hp[j]); hmO = max(hp[j], E[j+1])
    (borders max(hp, 0) as small DVE tensor_scalar ops)
hm lands in rows 1..8 of a 10-row tile whose rows 0/9 are partition-halo rows
(zero padding at image edges), then vertical is a pure pair trick:
    vp[q] = max(hmx[2q], hmx[2q+1]);  vm[2s] = max(vp[s], hmx[2s+2]);
    vm[2s+1] = max(hmx[2s+1], vp[s+1])
Halos either via PE shift-matmul -> PSUM -> ACT copy (HALO_DMA=False) or via
an SBUF->SBUF partition-shifted DMA on the otherwise-idle sync ring
(HALO_DMA=True; partition 0/127 halo rows zeroed by gpsimd).

Measured scheduling facts baked in: first DMA bytes land ~8.1 us regardless
of queue (global bring-up); the scalar (ACT) HWDGE ring sustains ~330 GB/s
vs sync's ~190, so all loads ride scalar with image 0 in three row-chunks
(DVE starts ~12 us on the first 512 KiB); emission interleaves image i's
vertical stage after image i+1's horizontal stage so halo latency hides
under DVE work; stores are split (halves, last image quarters) so the final
store tail is ~1.5 us.
"""

import sys

sys.path.insert(0, "/opt/trn_rl_repo")

import numpy as np

N_CORES = 8
IMGS_PER_CORE = 3
H = W = 1024
R = 8  # rows per partition
P = 128
J = 512  # half-row (even/odd) length
FREE = R * 2 * J  # 8192 elems per partition per image

IMG0_CHUNKS = [(0, 2), (2, 4), (4, 6), (6, 8)]
HALO_DMA = "--halo-dma" in sys.argv

_COMPILED_NC = None


def _build_nc(halo_dma=None):
    import concourse.mybir as mybir
    import concourse.tile as tile
    from concourse import bacc

    if halo_dma is None:
        halo_dma = HALO_DMA

    f16 = mybir.dt.float16
    f32 = mybir.dt.float32
    MAX = mybir.AluOpType.max

    nc = bacc.Bacc(None)
    x = nc.declare_dram_parameter("input", [IMGS_PER_CORE, P, FREE], f16, isOutput=False)
    y = nc.declare_dram_parameter("output", [IMGS_PER_CORE, P, FREE], f16, isOutput=True)

    with tile.TileContext(nc) as tc:
        with (
            tc.tile_pool(name="io", bufs=6) as io,
            tc.tile_pool(name="tmp", bufs=6) as tmp,
            tc.tile_pool(name="hmp", bufs=3) as hmp,
            tc.tile_pool(name="shp", bufs=1) as shp,
            tc.tile_pool(name="psum", bufs=2, space="PSUM") as psp,
        ):
            # Shifted identities (fp16) on the otherwise-idle Pool engine.
            sdn = shp.tile([P, P], f16, tag="sdn")
            sup = shp.tile([P, P], f16, tag="sup")
            for t, base in ((sdn, 1), (sup, -1)):
                nc.gpsimd.memset(t[:], 0.0)
                nc.gpsimd.affine_select(
                    out=t[:],
                    in_=t[:],
                    compare_op=mybir.AluOpType.not_equal,
                    fill=1.0,
                    base=base,
                    pattern=[[-1, P]],
                    channel_multiplier=1,
                )

            X = [None] * IMGS_PER_CORE
            hmx = [None] * IMGS_PER_CORE

            # ALL loads on the scalar (fast, ~330 GB/s) ring, in consumption
            # order. A second concurrent queue would starve image 0's small
            # chunks (DMA engines round-robin per DESCRIPTOR, so 16 KiB
            # descriptors on another queue get 4x the bytes of these 4 KiB
            # ones). One queue delivers bytes exactly in program order, and
            # the v-stages between h-stages give the ring time to stay ahead.
            for i in range(IMGS_PER_CORE):
                X[i] = io.tile([P, FREE], f16, tag="io", name=f"X{i}")
                if i == 0:
                    for r0, r1 in IMG0_CHUNKS:
                        c0, c1 = r0 * 2 * J, r1 * 2 * J
                        nc.scalar.dma_start(out=X[i][:, c0:c1], in_=x[i][:, c0:c1])
                else:
                    nc.scalar.dma_start(out=X[i][:], in_=x[i][:])

            def h_stage(i, chunks):
                X3 = X[i][:].rearrange("p (r e j) -> p r e j", e=2, j=J)
                hp = tmp.tile([P, R * J], f16, tag="tmp", name=f"hp{i}")
                hp3 = hp[:].rearrange("p (r j) -> p r j", j=J)
                hmx[i] = hmp.tile([P, 10 * 2 * J], f16, tag="hm", name=f"hmx{i}")
                h3 = hmx[i][:].rearrange("p (r e j) -> p r e j", e=2, j=J)
                hmf = hmx[i][:].rearrange("p (r c) -> p r c", c=2 * J)
                hpf = hp[:].rearrange("p (r c) -> p r c", c=J)
                for r0, r1 in chunks:
                    nc.vector.tensor_tensor(
                        out=hp3[:, r0:r1],
                        in0=X3[:, r0:r1, 0],
                        in1=X3[:, r0:r1, 1],
                        op=MAX,
                    )
                    nc.vector.tensor_tensor(
                        out=h3[:, r0 + 1 : r1 + 1, 0, 1:J],
                        in0=X3[:, r0:r1, 1, 0 : J - 1],
                        in1=hp3[:, r0:r1, 1:J],
                        op=MAX,
                    )
                    nc.vector.tensor_tensor(
                        out=h3[:, r0 + 1 : r1 + 1, 1, 0 : J - 1],
                        in0=hp3[:, r0:r1, 0 : J - 1],
                        in1=X3[:, r0:r1, 0, 1:J],
                        op=MAX,
                    )
                # both zero-pad border columns (E col 0 at flat 0, O col J-1
                # at flat 2J-1) in ONE strided max-with-0 per image on the
                # idle Pool engine: a single producer means the vp op that
                # reads the border cols carries one semaphore wait instead of
                # one per h-chunk (4 for image 0). (On ACT this would hoist a
                # ~1.3us activation-table load ahead of the load DMA issues;
                # in-stream on DVE it costs more than the waits it removes.)
                nc.gpsimd.tensor_scalar_max(
                    out=hmf[:, 1 : R + 1, 0 : 2 * J : 2 * J - 1],
                    in0=hpf[:, 0:R, 0 : J : J - 1],
                    scalar1=0.0,
                )

            def halo_stage(i):
                hm = hmx[i]
                row1 = 1 * 2 * J
                row8 = 8 * 2 * J
                row9 = 9 * 2 * J
                if halo_dma:
                    # hmx row 0 of partition p = hm row 8 of partition p-1;
                    # hmx row 9 of partition p = hm row 1 of partition p+1.
                    # Border partitions get the zero pad from the zero tile.
                    nc.sync.dma_start(out=hm[0:1, 0 : 2 * J], in_=zt[0:1, :])
                    nc.sync.dma_start(
                        out=hm[P - 1 : P, row9 : row9 + 2 * J],
                        in_=zt[P - 1 : P, :],
                    )
                    nc.sync.dma_start(
                        out=hm[1:P, 0 : 2 * J],
                        in_=hm[0 : P - 1, row8 : row8 + 2 * J],
                    )
                    nc.sync.dma_start(
                        out=hm[0 : P - 1, row9 : row9 + 2 * J],
                        in_=hm[1:P, row1 : row1 + 2 * J],
                    )
                else:
                    ps = psp.tile([P, 2 * 2 * J], f32, tag="ps")  # [dh | uh]
                    for c0 in (0, J):
                        nc.tensor.matmul(
                            ps[:, 2 * J + c0 : 2 * J + c0 + J],
                            sup[:],
                            hm[:, row1 + c0 : row1 + c0 + J],
                            start=True,
                            stop=True,
                        )
                    for c0 in (0, J):
                        nc.tensor.matmul(
                            ps[:, c0 : c0 + J],
                            sdn[:],
                            hm[:, row8 + c0 : row8 + c0 + J],
                            start=True,
                            stop=True,
                        )
                    nc.scalar.copy(out=hm[:, 0 : 2 * J], in_=ps[:, 0 : 2 * J])
                    nc.scalar.copy(out=hm[:, row9:], in_=ps[:, 2 * J :])

            def v_stage(i, last):
                # vp anchored on interior pairs (1,2)(3,4)(5,6)(7,8): vp needs
                # no halo rows (shorter cross-engine chain) and the vertical
                # stage totals 12288 elems instead of 13312.
                #   vm[2s]   = max(hmx[2s],   vp[s])
                #   vm[2s+1] = max(vp[s], hmx[2s+3])
                # vm writes de-interleaved row blocks (even rows 0,2,4,6 in
                # the first half of the tile, odd rows 1,3,5,7 in the second;
                # host re-interleaves in _unpack). Fully contiguous outputs
                # dodge the DVE write-path penalty measured on 8 KB-strided
                # out APs at count 4 (~2.4x slower than 2x_1p), letting each
                # image's vm run as one full-width op per parity: 8 fewer
                # DVE instructions than per-(0,4)-group pairs.
                h3 = hmx[i][:].rearrange("p (r e j) -> p r e j", e=2, j=J)
                vp = tmp.tile([P, 4 * 2 * J], f16, tag="tmp", name=f"vp{i}")
                vp3 = vp[:].rearrange("p (q e j) -> p q e j", e=2, j=J)
                vm = io.tile([P, FREE], f16, tag="io")
                # [b (even|odd block), q (4 rows), e, j]
                vm4 = vm[:].rearrange("p (b q e j) -> p b q e j", b=2, e=2, j=J)
                nc.vector.tensor_tensor(
                    out=vp3[:], in0=h3[:, 1:9:2], in1=h3[:, 2:10:2], op=MAX
                )
                halves = [(0, 2), (2, 4)] if last else [(0, 4)]
                # even rows: vm[2q] = max(hmx[2q], vp[q])
                for q0, q1 in halves:
                    nc.vector.tensor_tensor(
                        out=vm4[:, 0, q0:q1],
                        in0=h3[:, 2 * q0 : 2 * q1 : 2],
                        in1=vp3[:, q0:q1],
                        op=MAX,
                    )
                # odd rows: vm[2q+1] = max(vp[q], hmx[2q+3])
                for q0, q1 in halves:
                    nc.vector.tensor_tensor(
                        out=vm4[:, 1, q0:q1],
                        in0=vp3[:, q0:q1],
                        in1=h3[:, 2 * q0 + 3 : 2 * q1 + 2 : 2],
                        op=MAX,
                    )
                # stores: one contiguous transfer per parity block (halves
                # for the last image; its final half rides scalar after the
                # last DVE op, the sync one fires earlier)
                if not last:
                    eng = "sync" if i == 0 else "scalar"
                    for b in (0, 1):
                        getattr(nc, eng).dma_start(
                            out=y[i][:, b * 4 * 2 * J : (b + 1) * 4 * 2 * J],
                            in_=vm[:, b * 4 * 2 * J : (b + 1) * 4 * 2 * J],
                        )
                else:
                    for (b, q0, q1), eng in (
                        ((0, 0, 2), "scalar"),
                        ((0, 2, 4), "scalar"),
                        ((1, 0, 2), "sync"),
                        ((1, 2, 4), "scalar"),
                    ):
                        c0 = (b * 4 + q0) * 2 * J
                        c1 = (b * 4 + q1) * 2 * J
                        getattr(nc, eng).dma_start(
                            out=y[i][:, c0:c1], in_=vm[:, c0:c1]
                        )

            h_stage(0, IMG0_CHUNKS)
            halo_stage(0)
            v_stage(0, last=False)
            h_stage(1, [(0, 8)])
            halo_stage(1)
            v_stage(1, last=False)
            h_stage(2, [(0, 8)])
            halo_stage(2)
            v_stage(2, last=True)

    nc.compile()
    return nc


def _get_nc():
    global _COMPILED_NC
    if _COMPILED_NC is None:
        _COMPILED_NC = _build_nc()
    return _COMPILED_NC


def _pack(input):
    # (n, H, W) f32 -> (n, P, FREE) f16, per-partition [row][even|odd]
    x16 = input.reshape(-1, P, R, J, 2).astype(np.float16)
    return np.ascontiguousarray(x16.transpose(0, 1, 2, 4, 3)).reshape(-1, P, FREE)


def _unpack(out):
    # (n, P, FREE) f16 -> (n, H, W) f32.
    # Device layout per partition: [even rows 0,2,4,6 | odd rows 1,3,5,7],
    # each row [E cols | O cols]; re-interleave both here.
    o = out.reshape(-1, P, 2, 4, 2, J).transpose(0, 1, 3, 2, 5, 4)
    return np.ascontiguousarray(o).reshape(-1, H, W).astype(np.float32)


def _reference_fallback(input, se):
    se = np.asarray(se, dtype=np.float32)
    se_h, se_w = se.shape
    pad_h, pad_w = se_h // 2, se_w // 2
    se_m1 = (se - 1.0).reshape(-1)
    padded = np.pad(input, ((0, 0), (0, 0), (pad_h, pad_h), (pad_w, pad_w)))
    out = None
    Hh, Ww = input.shape[2], input.shape[3]
    for i in range(se_h * se_w):
        xs, ys = i // se_h, i % se_h
        mask = np.float32(1.0) if se_m1[i] >= 0 else np.float32(0.0)
        contrib = mask * padded[:, :, xs : xs + Hh, ys : ys + Ww] + se_m1[i]
        out = contrib if out is None else np.maximum(out, contrib)
    return out


def kernel(input, se):
    from concourse.bass_utils import run_bass_kernel_spmd

    input = np.ascontiguousarray(np.asarray(input, dtype=np.float32))
    se_np = np.asarray(se, dtype=np.float32)
    if se_np.shape != (3, 3) or not np.all(se_np == 1.0) or input.shape != (
        8,
        3,
        H,
        W,
    ):
        return _reference_fallback(input, se_np).astype(np.float32)

    nc = _get_nc()
    flat = _pack(input.reshape(N_CORES * IMGS_PER_CORE, H, W))
    in_maps = [
        {"input": flat[k * IMGS_PER_CORE : (k + 1) * IMGS_PER_CORE]}
        for k in range(N_CORES)
    ]
    last_err = None
    for _attempt in range(3):
        try:
            res = run_bass_kernel_spmd(nc, in_maps, list(range(N_CORES)))
            out = np.concatenate(
                [res.results[k]["output"] for k in range(N_CORES)], axis=0
            )
            return _unpack(out).reshape(8, 3, H, W)
        except Exception as e:  # transient NRT_EXEC_UNIT_UNRECOVERABLE etc.
            last_err = e
    raise last_err

